# revision 1
# baseline (speedup 1.0000x reference)
"""GATv2 attention head (kgcnn AttentionHeadGATV2) on 8 Trainium2 NeuronCores.

Strategy (edge/graph parallelism, self-contained — no file reads):
  * Host: bucket edges by receiver node; core c owns receiver nodes
    [c*6250, (c+1)*6250). Within a core, edges are grouped into 49
    receiver "windows" of 128 nodes, each padded to a fixed capacity
    (split into send<32768 / send>=32768 halves so gather indices fit
    int16). All index/metadata arrays are precomputed on host.
  * Device precompute (replicated per core): psw = [P_s | w_n] where
    P_s = node @ (W_att[128:]*c)[:,perm],  w_n = node @ W_lin,
    and a per-core shard of P_r = node @ (W_att[:128]*c)[:,perm] + b_hat.
    The column scaling c and permutation fold a_vec into the leaky-relu
    (sign trick), so the attention logit becomes a plain row-sum.
    b_lin is folded in at the end (sum of attention weights is 1).
  * Device edge phase, per window: dma_gather P_r[recv], [P_s|w_n][send]
    (bf16, <=1024 idxs per gather); z = pr + ps; h = lrelu_{0.2/5}(z)
    via (z*a) max/min z on DVE (column split at k); a = rowsum(h) via
    per-subtile tensor_scalar accumulate; ex = exp(a) via tanh identity;
    one-hot (x ex) built by fused tensor_scalar(is_equal, mult);
    scatter-add via PE matmul into PSUM [128 nodes x 128] (+denominator
    column); out = lrelu_{0.2}(numer/denom + b_lin).
"""

import sys

sys.path.insert(0, "/opt/trn_rl_repo")

import numpy as np
import ml_dtypes

import concourse.bacc as bacc
import concourse.bass as bass
import concourse.mybir as mybir
import concourse.tile as tile
from concourse import bass_utils

DT = mybir.dt
ALU = mybir.AluOpType
ACTF = mybir.ActivationFunctionType
AXIS = mybir.AxisListType

BF16 = ml_dtypes.bfloat16

# Problem constants (hardcoded per the task contract).
N_NODES = 50000
N_EDGES = 800000
F_DIM = 128
UNITS = 128
ALPHA = 0.2
NCORES = 8
NPC = N_NODES // NCORES          # 6250 receiver nodes per core
WIN = 128                        # receiver-window size (PSUM partition dim)
NW = (NPC + WIN - 1) // WIN      # 49 windows per core
LAST_ROWS = NPC - (NW - 1) * WIN # 106 rows in the last window
ST_ROWS = 2048                   # precompute supertile rows
NODE_PAD = 51200                 # 25 * 2048
SHARD_PAD = 8192                 # 4 * 2048
LO_BASE = 32768                  # int16 gather-index split point
GMAX = 1024                      # max idxs per dma_gather (SWDGE ring limit)

_BUILD_CACHE = {}

DEFAULT_CFG = dict(
    n_nodes=N_NODES, n_edges=N_EDGES, ncores=NCORES, npc=NPC, nw=NW,
    last_rows=LAST_ROWS, node_pad=NODE_PAD, shard_pad=SHARD_PAD,
    lo_base=LO_BASE,
)


def _build(cap_lo, cap_hi, k_pos, cfg=None, compile=True, dbg=False):
    p = dict(DEFAULT_CFG)
    if cfg is not None:
        p.update(cfg)

    nsub_lo = cap_lo // 128
    nsub_hi = cap_hi // 128
    cap = cap_lo + cap_hi
    nsub = nsub_lo + nsub_hi
    cap16 = cap // 16
    lo16 = cap_lo // 16
    hi16 = cap_hi // 16
    nw = p["nw"]

    nc = bacc.Bacc("TRN2", target_bir_lowering=False, debug=False,
                   num_devices=p["ncores"], num_swdge_queues=4)

    node_pad_d = nc.dram_tensor("node_pad", [p["node_pad"], 128], DT.bfloat16,
                                kind="ExternalInput")
    node_shard_d = nc.dram_tensor("node_shard", [p["shard_pad"], 128],
                                  DT.bfloat16, kind="ExternalInput")
    w_s_d = nc.dram_tensor("w_s", [128, 128], DT.bfloat16, kind="ExternalInput")
    w_lin_d = nc.dram_tensor("w_lin", [128, 128], DT.bfloat16,
                             kind="ExternalInput")
    w_r_d = nc.dram_tensor("w_r", [128, 128], DT.bfloat16, kind="ExternalInput")
    blin_d = nc.dram_tensor("blin", [128, 128], DT.float32,
                            kind="ExternalInput")
    bias_r_d = nc.dram_tensor("bias_r", [128, 128], DT.float32,
                              kind="ExternalInput")
    iota_d = nc.dram_tensor("iota", [128, 128], DT.bfloat16,
                            kind="ExternalInput")
    recvidx_d = nc.dram_tensor("recvidx", [128, nw * cap16], DT.int16,
                               kind="ExternalInput")
    sendlo_d = nc.dram_tensor("sendlo", [128, nw * lo16], DT.int16,
                              kind="ExternalInput")
    sendhi_d = nc.dram_tensor("sendhi", [128, nw * hi16], DT.int16,
                              kind="ExternalInput")
    recvadj_d = nc.dram_tensor("recvadj", [128, nw * nsub], DT.float32,
                               kind="ExternalInput")
    out_d = nc.dram_tensor("out", [p["npc"], 128], DT.float32,
                           kind="ExternalOutput")
    if dbg:
        dbg_psw = nc.dram_tensor("dbg_psw", [p["node_pad"], 256], DT.bfloat16,
                                 kind="ExternalOutput")
        dbg_pr = nc.dram_tensor("dbg_pr", [p["shard_pad"], 128], DT.bfloat16,
                                kind="ExternalOutput")

    with tile.TileContext(nc) as tc:
        with (
            tc.tile_pool(name="consts", bufs=1) as cpool,
            tc.tile_pool(name="dram", bufs=1, space="DRAM") as dpool,
        ):
            psw_dram = dpool.tile([p["node_pad"], 256], DT.bfloat16)
            pr_dram = dpool.tile([p["shard_pad"], 128], DT.bfloat16)

            ws_sb = cpool.tile([128, 128], DT.bfloat16)
            nc.sync.dma_start(ws_sb[:], w_s_d[:])
            wlin_sb = cpool.tile([128, 128], DT.bfloat16)
            nc.sync.dma_start(wlin_sb[:], w_lin_d[:])
            wr_sb = cpool.tile([128, 128], DT.bfloat16)
            nc.sync.dma_start(wr_sb[:], w_r_d[:])
            blin_sb = cpool.tile([128, 128], DT.float32)
            nc.sync.dma_start(blin_sb[:], blin_d[:])
            biasr_sb = cpool.tile([128, 128], DT.float32)
            nc.sync.dma_start(biasr_sb[:], bias_r_d[:])
            iota_sb = cpool.tile([128, 128], DT.bfloat16)
            nc.sync.dma_start(iota_sb[:], iota_d[:])
            recvidx_sb = cpool.tile([128, nw * cap16], DT.int16)
            nc.sync.dma_start(recvidx_sb[:], recvidx_d[:])
            sendlo_sb = cpool.tile([128, nw * lo16], DT.int16)
            nc.sync.dma_start(sendlo_sb[:], sendlo_d[:])
            sendhi_sb = cpool.tile([128, nw * hi16], DT.int16)
            nc.sync.dma_start(sendhi_sb[:], sendhi_d[:])
            recvadj_sb = cpool.tile([128, nw * nsub], DT.float32)
            nc.sync.dma_start(recvadj_sb[:], recvadj_d[:])
            ones_sb = cpool.tile([128, 1], DT.bfloat16)
            nc.vector.memset(ones_sb[:], 1.0)

            # ---------------- precompute phase ----------------
            nsubt = ST_ROWS // 128
            with (
                tc.tile_pool(name="pcsb", bufs=3) as pc,
                tc.tile_pool(name="pcpsum", bufs=6, space="PSUM") as pcp,
            ):
                for st in range(p["node_pad"] // ST_ROWS):
                    ntile = pc.tile([128, ST_ROWS], DT.bfloat16, tag="nodeT")
                    nc.sync.dma_start(
                        ntile[:], node_pad_d[st * ST_ROWS:(st + 1) * ST_ROWS, :],
                        transpose=True)
                    rows = pc.tile([128, nsubt, 256], DT.bfloat16, tag="pswrow")
                    for j in range(nsubt):
                        ps = pcp.tile([128, 256], DT.float32, tag="pcps")
                        lhsT = ntile[:, j * 128:(j + 1) * 128]
                        nc.tensor.matmul(ps[:, 0:128], lhsT, ws_sb[:],
                                         start=True, stop=True)
                        nc.tensor.matmul(ps[:, 128:256], lhsT, wlin_sb[:],
                                         start=True, stop=True)
                        nc.scalar.copy(rows[:, j, :], ps[:])
                    r0 = st * ST_ROWS
                    nc.sync.dma_start(psw_dram[r0:r0 + ST_ROWS, :], rows[:])
                for st in range(p["shard_pad"] // ST_ROWS):
                    ntile = pc.tile([128, ST_ROWS], DT.bfloat16, tag="nodeT")
                    nc.sync.dma_start(
                        ntile[:],
                        node_shard_d[st * ST_ROWS:(st + 1) * ST_ROWS, :],
                        transpose=True)
                    rows2 = pc.tile([128, nsubt, 128], DT.bfloat16, tag="prrow")
                    for j in range(nsubt):
                        ps = pcp.tile([128, 256], DT.float32, tag="pcps")
                        lhsT = ntile[:, j * 128:(j + 1) * 128]
                        nc.tensor.matmul(ps[:, 0:128], lhsT, wr_sb[:],
                                         start=True, stop=True)
                        nc.vector.tensor_tensor(rows2[:, j, :], ps[:, 0:128],
                                                biasr_sb[:], ALU.add)
                    r0 = st * ST_ROWS
                    nc.sync.dma_start(pr_dram[r0:r0 + ST_ROWS, :], rows2[:])

            if dbg:
                nc.sync.dma_start(dbg_psw[:], psw_dram[:])
                nc.sync.dma_start(dbg_pr[:], pr_dram[:])

            # ---------------- edge phase ----------------
            with (
                tc.tile_pool(name="edge", bufs=3) as ep,
                tc.tile_pool(name="edge4", bufs=4) as ep4,
                tc.tile_pool(name="edge5", bufs=5) as ep5,
                tc.tile_pool(name="small", bufs=4) as sp,
                tc.tile_pool(name="epsum", bufs=4, space="PSUM") as pp,
            ):
                for w in range(nw):
                    gpr = ep4.tile([128, nsub, 128], DT.bfloat16, tag="gpr")
                    gmax = globals().get("GMAX_OVERRIDE", GMAX)
                    for g0 in range(0, cap, gmax):
                        gn = min(gmax, cap - g0)
                        nc.gpsimd.dma_gather(
                            gpr[:, g0 // 128:(g0 + gn) // 128, :], pr_dram[:],
                            recvidx_sb[:, w * cap16 + g0 // 16:
                                       w * cap16 + (g0 + gn) // 16],
                            gn, gn, 128, queue_num=0)
                    gpsw = ep5.tile([128, nsub, 256], DT.bfloat16, tag="gpsw")
                    for g0 in range(0, cap_lo, gmax):
                        gn = min(gmax, cap_lo - g0)
                        nc.gpsimd.dma_gather(
                            gpsw[:, g0 // 128:(g0 + gn) // 128, :], psw_dram[:],
                            sendlo_sb[:, w * lo16 + g0 // 16:
                                      w * lo16 + (g0 + gn) // 16],
                            gn, gn, 256, queue_num=0)
                    for g0 in range(0, cap_hi, gmax):
                        gn = min(gmax, cap_hi - g0)
                        nc.gpsimd.dma_gather(
                            gpsw[:, nsub_lo + g0 // 128:
                                 nsub_lo + (g0 + gn) // 128, :],
                            psw_dram[p["lo_base"]:, :],
                            sendhi_sb[:, w * hi16 + g0 // 16:
                                      w * hi16 + (g0 + gn) // 16],
                            gn, gn, 256, queue_num=0)

                    z = ep.tile([128, nsub, 128], DT.bfloat16, tag="z")
                    nc.vector.tensor_tensor(z[:], gpr[:], gpsw[:, :, 0:128],
                                            ALU.add)
                    h = ep.tile([128, nsub, 128], DT.bfloat16, tag="h")
                    if k_pos > 0:
                        nc.vector.scalar_tensor_tensor(
                            h[:, :, 0:k_pos], z[:, :, 0:k_pos], ALPHA,
                            z[:, :, 0:k_pos], ALU.mult, ALU.max)
                    if k_pos < 128:
                        nc.vector.scalar_tensor_tensor(
                            h[:, :, k_pos:128], z[:, :, k_pos:128], 1.0 / ALPHA,
                            z[:, :, k_pos:128], ALU.mult, ALU.min)
                    a = sp.tile([128, nsub], DT.float32, tag="a")
                    for s in range(nsub):
                        nc.vector.tensor_scalar(h[:, s, :], h[:, s, :], 1.0,
                                                None, ALU.mult, ALU.add,
                                                accum_out=a[:, s:s + 1])
                    # exp(a) = (1 + tanh(a/2)) / (1 - tanh(a/2))
                    t = sp.tile([128, nsub], DT.float32, tag="t")
                    nc.scalar.activation(t[:], a[:], ACTF.Tanh, scale=0.5)
                    d = sp.tile([128, nsub], DT.float32, tag="d")
                    nc.vector.tensor_scalar(d[:], t[:], -1.0, 1.0, ALU.mult,
                                            ALU.add)
                    rd = sp.tile([128, nsub], DT.float32, tag="rd")
                    nc.vector.reciprocal(rd[:], d[:])
                    ex = sp.tile([128, nsub], DT.float32, tag="ex")
                    nc.vector.scalar_tensor_tensor(ex[:], t[:], 1.0, rd[:],
                                                   ALU.add, ALU.mult)

                    oh = ep4.tile([128, nsub, 128], DT.bfloat16, tag="oh")
                    for s in range(nsub):
                        nc.vector.tensor_scalar(
                            oh[:, s, :], iota_sb[:],
                            recvadj_sb[:, w * nsub + s:w * nsub + s + 1],
                            ex[:, s:s + 1], ALU.is_equal, ALU.mult)

                    pw_n = pp.tile([128, 128], DT.float32, tag="pwn")
                    pw_d = pp.tile([128, 1], DT.float32, tag="pwd")
                    for s in range(nsub):
                        nc.tensor.matmul(pw_n[:], oh[:, s, :],
                                         gpsw[:, s, 128:256],
                                         start=(s == 0), stop=(s == nsub - 1),
                                         skip_group_check=True)
                        nc.tensor.matmul(pw_d[:], oh[:, s, :],
                                         ones_sb[:],
                                         start=(s == 0), stop=(s == nsub - 1),
                                         skip_group_check=True)

                    dn = sp.tile([128, 1], DT.float32, tag="dn")
                    nc.vector.tensor_scalar(dn[:], pw_d[:], 1e-30, None,
                                            ALU.add)
                    rn = sp.tile([128, 1], DT.float32, tag="rn")
                    nc.vector.reciprocal(rn[:], dn[:])
                    o1 = sp.tile([128, 128], DT.float32, tag="o1")
                    nc.vector.scalar_tensor_tensor(o1[:], pw_n[:], rn[:],
                                                   blin_sb[:], ALU.mult,
                                                   ALU.add)
                    o2 = sp.tile([128, 128], DT.float32, tag="o2")
                    nc.vector.scalar_tensor_tensor(o2[:], o1[:], ALPHA, o1[:],
                                                   ALU.mult, ALU.max)
                    rows = WIN if w < nw - 1 else p["last_rows"]
                    nc.sync.dma_start(out_d[w * WIN:w * WIN + rows, :],
                                      o2[0:rows, :])

    if compile:
        nc.compile()
    return nc


def _store_perm(i):
    """Map a logical row index to its stored row in psw/pr DRAM.

    The precompute writes SBUF tiles [128, J, 256] with one dma_start per
    ST_ROWS block; the DMA lays out (partition p, subtile j) at block row
    p*J + j, while logical row r = j*128 + p. Gather indices must follow."""
    J = ST_ROWS // 128
    b = i // ST_ROWS
    r = i % ST_ROWS
    return b * ST_ROWS + (r % 128) * J + (r // 128)


def _wrap16(arr):
    """[nw, capx] int16 slot-index array -> [128, nw*capx//16] wrapped and
    replicated across the 8 GPSIMD core partition groups."""
    nwn, capx = arr.shape
    w = arr.reshape(nwn, capx // 16, 16).transpose(2, 0, 1).reshape(
        16, nwn * (capx // 16))
    return np.tile(w, (8, 1)).copy()


def _prep(node, edge_index, W_lin, b_lin, W_att, b_att, a_vec, cfg=None):
    """Host-side sharding/preprocessing. Returns (in_maps, cap_lo, cap_hi, k)."""
    p = dict(DEFAULT_CFG)
    if cfg is not None:
        p.update(cfg)
    ncores, npc, nw = p["ncores"], p["npc"], p["nw"]
    node_pad_n, shard_pad_n, lo_base = (p["node_pad"], p["shard_pad"],
                                        p["lo_base"])
    n_nodes = p["n_nodes"]

    recv = np.asarray(edge_index[0], dtype=np.int64)
    send = np.asarray(edge_index[1], dtype=np.int64)

    # ---- fold a_vec into the weights (sign trick) ----
    sa = np.asarray(a_vec[:, 0], dtype=np.float32)
    pos = sa >= 0
    perm = np.concatenate([np.where(pos)[0], np.where(~pos)[0]])
    k = int(pos.sum())
    cvec = np.where(pos, sa, ALPHA * sa).astype(np.float32)

    W_att = np.asarray(W_att, dtype=np.float32)
    w_r_hat = (W_att[:F_DIM, :] * cvec[None, :])[:, perm]
    w_s_hat = (W_att[F_DIM:, :] * cvec[None, :])[:, perm]
    b_hat = (np.asarray(b_att, np.float32) * cvec)[perm]

    # ---- edge bucketing ----
    cid = recv // npc
    rloc = recv - cid * npc
    wid = rloc >> 7
    grp = (send >= lo_base).astype(np.int64)
    key = (cid * nw + wid) * 2 + grp
    order = np.lexsort((send, rloc, key))
    ks, rs, ss = key[order], rloc[order], send[order]
    counts = np.bincount(key, minlength=ncores * nw * 2).reshape(ncores, nw, 2)
    starts = np.concatenate([[0], np.cumsum(counts.reshape(-1))])[:-1].reshape(
        ncores, nw, 2)

    cap_lo = int(np.ceil(counts[:, :, 0].max() / 128) * 128)
    cap_hi = int(np.ceil(max(counts[:, :, 1].max(), 1) / 128) * 128)
    cap = cap_lo + cap_hi
    nsub = cap // 128

    recvidx = np.zeros((ncores, nw, cap), np.int16)
    recvadj = np.full((ncores, nw, cap), -1000.0, np.float32)
    sendlo = np.zeros((ncores, nw, cap_lo), np.int16)
    sendhi = np.zeros((ncores, nw, cap_hi), np.int16)
    for c in range(ncores):
        for w in range(nw):
            nlo = counts[c, w, 0]
            b0 = starts[c, w, 0]
            recvidx[c, w, :nlo] = _store_perm(rs[b0:b0 + nlo])
            recvadj[c, w, :nlo] = rs[b0:b0 + nlo] - w * WIN
            sendlo[c, w, :nlo] = _store_perm(ss[b0:b0 + nlo])
            nhi = counts[c, w, 1]
            b1 = starts[c, w, 1]
            recvidx[c, w, cap_lo:cap_lo + nhi] = _store_perm(rs[b1:b1 + nhi])
            recvadj[c, w, cap_lo:cap_lo + nhi] = rs[b1:b1 + nhi] - w * WIN
            sendhi[c, w, :nhi] = _store_perm(ss[b1:b1 + nhi]) - lo_base

    # ---- node arrays ----
    node = np.asarray(node, dtype=np.float32)
    node_pad = np.zeros((node_pad_n, F_DIM), BF16)
    node_pad[:n_nodes] = node.astype(BF16)

    iota = np.tile(np.arange(128, dtype=np.float32), (128, 1)).astype(BF16)
    blin = np.tile(np.asarray(b_lin, np.float32)[None, :], (128, 1))
    bias_r = np.tile(b_hat[None, :], (128, 1)).astype(np.float32)

    in_maps = []
    for c in range(ncores):
        shard = np.zeros((shard_pad_n, F_DIM), BF16)
        shard[:npc] = node[c * npc:(c + 1) * npc].astype(BF16)
        adj = recvadj[c].reshape(nw, nsub, 128).transpose(2, 0, 1).reshape(
            128, nw * nsub).copy()
        in_maps.append({
            "node_pad": node_pad,
            "node_shard": shard,
            "w_s": w_s_hat.astype(BF16),
            "w_lin": np.asarray(W_lin, np.float32).astype(BF16),
            "w_r": w_r_hat.astype(BF16),
            "blin": blin,
            "bias_r": bias_r,
            "iota": iota,
            "recvidx": _wrap16(recvidx[c]),
            "sendlo": _wrap16(sendlo[c]),
            "sendhi": _wrap16(sendhi[c]),
            "recvadj": adj,
        })
    return in_maps, cap_lo, cap_hi, k


def kernel(node, edge, edge_index, W_lin, b_lin, W_att, b_att, a_vec):
    in_maps, cap_lo, cap_hi, k = _prep(node, edge_index, W_lin, b_lin,
                                       W_att, b_att, a_vec)
    ckey = (cap_lo, cap_hi, k)
    if ckey not in _BUILD_CACHE:
        _BUILD_CACHE[ckey] = _build(cap_lo, cap_hi, k)
    nc = _BUILD_CACHE[ckey]
    res = bass_utils.run_bass_kernel_spmd(nc, in_maps,
                                          core_ids=list(range(NCORES)))
    out = np.concatenate([res.results[c]["out"] for c in range(NCORES)],
                         axis=0)
    return np.ascontiguousarray(out[:N_NODES]).astype(np.float32)



# revision 10
# speedup vs baseline: 1.2531x; 1.2531x over previous
"""GATv2 attention head (kgcnn AttentionHeadGATV2) on 8 Trainium2 NeuronCores.

Strategy (edge/graph parallelism, self-contained — no file reads):
  * Host: bucket edges by receiver node; core c owns receivers
    [c*6250, (c+1)*6250), grouped into 49 windows of 128 receivers.
    Within a window edges are split by send row range (lo/hi tables so
    gather indices fit int16) and padded per 128-block to per-window
    capacities (max over the 8 cores, so one SPMD program serves all).
  * Device precompute: two DRAM tables.
      tlo = [ pr-block (8192 rows) | psw rows for send<LO_N ]
      thi = [ psw rows for send>=LO_N ]
    psw row = [P_s | w_n] (256 cols bf16), P_s = node @ (W_att[128:]*c)[:,perm],
    w_n = node @ W_lin;  pr row = [P_r | onehot(row%128)] where
    P_r = node_shard @ (W_att[:128]*c)[:,perm]; the onehot mask half is
    host-initialized, P_r half written by PE matmuls from a transposed
    node input. Column scaling c + permutation fold a_vec into the
    leaky-relu (sign trick) so the logit is a plain row-sum.
  * Edge phase per window w: TWO dma_gathers fill one [128, NS, 256]
    tile: recv block (pr rows: P_r + scatter mask) mirrors the send
    block (psw rows) slot-for-slot.  z = P_r + P_s (DVE);
    h = lrelu sign-split (DVE x2); a = tensor_reduce row-sum (DVE);
    exr = Exp(a) broadcast along units (Act, stride-0 input AP);
    oh = mask * exr (DVE);  PE scatter: pw += oh^T @ w_n, pwd += oh^T @ 1;
    out = Prelu(pw * (1/pwd)) on Act, stored bf16.
  * Padding slots gather table row ZROW (all zeros) => mask 0 => no
    contribution; b_lin / b_att are zero in the graded setup and folded
    via cheap conditional paths when nonzero.
"""

import sys

sys.path.insert(0, "/opt/trn_rl_repo")

import numpy as np
import ml_dtypes

import concourse.bacc as bacc
import concourse.bass as bass
import concourse.mybir as mybir
import concourse.tile as tile
from concourse import bass_utils
from concourse.bass_types import AP

DT = mybir.dt
ALU = mybir.AluOpType
ACTF = mybir.ActivationFunctionType
AXIS = mybir.AxisListType

BF16 = ml_dtypes.bfloat16

# Problem constants (hardcoded per the task contract).
N_NODES = 50000
N_EDGES = 800000
F_DIM = 128
UNITS = 128
ALPHA = 0.2
NCORES = 8
NPC = N_NODES // NCORES          # 6250 receiver nodes per core
WIN = 128                        # receiver-window size (PSUM partition dim)
NW = (NPC + WIN - 1) // WIN      # 49 windows per core
LAST_ROWS = NPC - (NW - 1) * WIN # 106 rows in the last window
CHUNK = 2048                     # precompute chunk rows
PR_ROWS = 8192                   # pr block rows (4 chunks), rows>=6250 zero
ZROW = 6250                      # all-zero table row used for padding slots
LO_N = 18432                     # sends < LO_N gather from tlo (9 chunks)
HI_CHUNKS = 16                   # 16*2048 = 32768 rows >= nodes 18432..50000
TLO_ROWS = PR_ROWS + LO_N        # 26624
THI_ROWS = HI_CHUNKS * CHUNK     # 32768
SCRATCH = 16384                  # SWDGE carveout: 4096 descs per queue
GMAX = 1024                      # max idxs per dma_gather instruction

_BUILD_CACHE = {}


def _build(wins, k_pos, has_ba, has_bl, compile=True):
    """wins: tuple of (ns_lo, ns_hi) per window."""
    nw = len(wins)
    tot16_1 = sum(128 * (2 * (nl + nh) - nh) for nl, nh in wins) // 16
    tot16_2 = sum(128 * nh for nl, nh in wins) // 16

    nc = bacc.Bacc("TRN2", target_bir_lowering=False, debug=False,
                   num_devices=NCORES, num_swdge_queues=4,
                   dynamic_dma_scratch_size=SCRATCH)

    nodeT_d = nc.dram_tensor("nodeT", [128, 25 * CHUNK], DT.bfloat16,
                             kind="ExternalInput")
    shardT_d = nc.dram_tensor("shardT", [128, PR_ROWS], DT.bfloat16,
                              kind="ExternalInput")
    wcat_d = nc.dram_tensor("wcat", [128, 256], DT.bfloat16,
                            kind="ExternalInput")
    wr_d = nc.dram_tensor("wr", [128, 128], DT.bfloat16, kind="ExternalInput")
    idx1_d = nc.dram_tensor("idx1", [128, tot16_1], DT.int16,
                            kind="ExternalInput")
    idx2_d = nc.dram_tensor("idx2", [128, tot16_2], DT.int16,
                            kind="ExternalInput")
    # tlo mask halves (and zero rows) are host-initialized; P_r/psw halves
    # overwritten on device.
    tlo_d = nc.dram_tensor("tlo", [TLO_ROWS, 256], DT.bfloat16,
                           kind="ExternalInput")
    thi_d = nc.dram_tensor("thi", [THI_ROWS, 256], DT.bfloat16,
                           kind="ExternalInput")
    if has_ba:
        bar_d = nc.dram_tensor("bar", [128, 256], DT.float32,
                               kind="ExternalInput")
    if has_bl:
        blin_d = nc.dram_tensor("blin", [128, 128], DT.float32,
                                kind="ExternalInput")
    out_d = nc.dram_tensor("out", [NPC, 128], DT.bfloat16,
                           kind="ExternalOutput")

    with tile.TileContext(nc) as tc:
        with tc.tile_pool(name="consts", bufs=1) as cpool:
            wcat_sb = cpool.tile([128, 256], DT.bfloat16)
            nc.sync.dma_start(wcat_sb[:], wcat_d[:])
            wr_sb = cpool.tile([128, 128], DT.bfloat16)
            nc.sync.dma_start(wr_sb[:], wr_d[:])
            idx1_sb = cpool.tile([128, tot16_1], DT.int16)
            nc.sync.dma_start(idx1_sb[:], idx1_d[:])
            idx2_sb = cpool.tile([128, tot16_2], DT.int16)
            nc.sync.dma_start(idx2_sb[:], idx2_d[:])
            ones_sb = cpool.tile([128, 1], DT.bfloat16)
            nc.vector.memset(ones_sb[:], 1.0)
            if has_ba:
                bar_sb = cpool.tile([128, 256], DT.float32)
                nc.sync.dma_start(bar_sb[:], bar_d[:])
            if has_bl:
                blin_sb = cpool.tile([128, 128], DT.float32)
                nc.sync.dma_start(blin_sb[:], blin_d[:])

            # ---------------- precompute phase ----------------
            J = CHUNK // 128
            with (
                tc.tile_pool(name="pcsb", bufs=3) as pc,
                tc.tile_pool(name="pcps", bufs=4, space="PSUM") as pcp,
            ):
                # psw chunks: 9 into tlo (after pr block), 16 into thi
                for st in range(9 + HI_CHUNKS):
                    ntile = pc.tile([128, CHUNK], DT.bfloat16, tag="nT")
                    nc.sync.dma_start(
                        ntile[:], nodeT_d[:, st * CHUNK:(st + 1) * CHUNK])
                    rows = pc.tile([128, J, 256], DT.bfloat16, tag="rows")
                    for j4 in range(J // 4):
                        ps = pcp.tile([128, 4, 256], DT.float32, tag="ps")
                        for i in range(4):
                            j = j4 * 4 + i
                            nc.tensor.matmul(ps[:, i, :],
                                             ntile[:, j * 128:(j + 1) * 128],
                                             wcat_sb[:], start=True, stop=True)
                        # PSUM->SBUF convert, alternating engines
                        if has_ba:
                            eng = nc.vector if j4 % 2 == 0 else nc.vector
                            nc.vector.tensor_tensor(
                                rows[:, j4 * 4:(j4 + 1) * 4, :], ps[:],
                                AP(bar_sb[:].tensor, bar_sb[:].offset,
                                   [bar_sb[:].ap[0], [0, 4], bar_sb[:].ap[1]]),
                                ALU.add)
                        elif j4 % 2 == 0:
                            nc.scalar.copy(rows[:, j4 * 4:(j4 + 1) * 4, :],
                                           ps[:])
                        else:
                            nc.vector.tensor_scalar(
                                rows[:, j4 * 4:(j4 + 1) * 4, :], ps[:], 1.0,
                                None, ALU.mult)
                    r0 = st * CHUNK
                    if st < 9:
                        nc.sync.dma_start(
                            tlo_d[PR_ROWS + r0:PR_ROWS + r0 + CHUNK, :],
                            rows[:])
                    else:
                        r0 = (st - 9) * CHUNK
                        nc.sync.dma_start(thi_d[r0:r0 + CHUNK, :], rows[:])
                # pr chunks: 4 into tlo rows [0, 8192), cols 0:128 only
                for st in range(PR_ROWS // CHUNK):
                    ntile = pc.tile([128, CHUNK], DT.bfloat16, tag="nT")
                    nc.sync.dma_start(
                        ntile[:], shardT_d[:, st * CHUNK:(st + 1) * CHUNK])
                    rows2 = pc.tile([128, J, 128], DT.bfloat16, tag="rows2")
                    for j4 in range(J // 4):
                        ps = pcp.tile([128, 4, 256], DT.float32, tag="ps")
                        for i in range(4):
                            j = j4 * 4 + i
                            nc.tensor.matmul(ps[:, i, 0:128],
                                             ntile[:, j * 128:(j + 1) * 128],
                                             wr_sb[:], start=True, stop=True)
                        if j4 % 2 == 0:
                            nc.scalar.copy(rows2[:, j4 * 4:(j4 + 1) * 4, :],
                                           ps[:, :, 0:128])
                        else:
                            nc.vector.tensor_scalar(
                                rows2[:, j4 * 4:(j4 + 1) * 4, :],
                                ps[:, :, 0:128], 1.0, None, ALU.mult)
                    r0 = st * CHUNK
                    nc.sync.dma_start(tlo_d[r0:r0 + CHUNK, 0:128], rows2[:])

            # ---------------- edge phase ----------------
            with (
                tc.tile_pool(name="egw", bufs=3) as egw,
                tc.tile_pool(name="ez", bufs=2) as ez,
                tc.tile_pool(name="eh", bufs=2) as eh,
                tc.tile_pool(name="eex", bufs=2) as eex,
                tc.tile_pool(name="eoh", bufs=2) as eoh,
                tc.tile_pool(name="esm", bufs=3) as esm,
                tc.tile_pool(name="eout", bufs=2) as eout,
                tc.tile_pool(name="eps", bufs=4, space="PSUM") as eps,
            ):
                off1 = 0
                off2 = 0
                for w in range(nw):
                    ns_lo, ns_hi = wins[w]
                    ns_es = ns_lo + ns_hi
                    NS = 2 * ns_es
                    n1 = 128 * (ns_es + ns_lo)
                    n2 = 128 * ns_hi
                    gw = egw.tile([128, NS, 256], DT.bfloat16, tag="gw")
                    qn = 2 * w
                    for g0 in range(0, n1, GMAX):
                        gn = min(GMAX, n1 - g0)
                        nc.gpsimd.dma_gather(
                            gw[:, g0 // 128:(g0 + gn) // 128, :], tlo_d[:],
                            idx1_sb[:, off1 + g0 // 16:off1 + (g0 + gn) // 16],
                            gn, gn, 256, queue_num=qn % 4)
                        qn += 1
                    base2 = ns_es + ns_lo
                    for g0 in range(0, n2, GMAX):
                        gn = min(GMAX, n2 - g0)
                        nc.gpsimd.dma_gather(
                            gw[:, base2 + g0 // 128:base2 + (g0 + gn) // 128,
                               :], thi_d[:],
                            idx2_sb[:, off2 + g0 // 16:off2 + (g0 + gn) // 16],
                            gn, gn, 256, queue_num=qn % 4)
                        qn += 1
                    off1 += n1 // 16
                    off2 += n2 // 16

                    z = ez.tile([128, ns_es, 128], DT.bfloat16, tag="z")
                    nc.vector.tensor_tensor(z[:], gw[:, 0:ns_es, 0:128],
                                            gw[:, ns_es:NS, 0:128], ALU.add)
                    # leaky-relu sign halves on the Activation engine:
                    # pos block = Prelu(z; 0.2), neg block = Prelu(z; 5.0)
                    h = eh.tile([128, ns_es, 128], DT.bfloat16, tag="h")
                    if k_pos > 0:
                        nc.scalar.activation(h[:, :, 0:k_pos],
                                             z[:, :, 0:k_pos], ACTF.Prelu,
                                             alpha=ALPHA)
                    if k_pos < 128:
                        nc.scalar.activation(h[:, :, k_pos:128],
                                             z[:, :, k_pos:128], ACTF.Prelu,
                                             alpha=1.0 / ALPHA)
                    # bf16 fold tree then fp32 reduce for the row-sum
                    f1 = ez.tile([128, ns_es, 64], DT.bfloat16, tag="f1")
                    nc.vector.tensor_tensor(f1[:], h[:, :, 0:64],
                                            h[:, :, 64:128], ALU.add)
                    f2 = ez.tile([128, ns_es, 32], DT.bfloat16, tag="f2")
                    nc.vector.tensor_tensor(f2[:], f1[:, :, 0:32],
                                            f1[:, :, 32:64], ALU.add)
                    a = esm.tile([128, ns_es], DT.float32, tag="a")
                    nc.vector.tensor_reduce(a[:], f2[:], AXIS.X, ALU.add)

                    # exr[p, s, :] = exp(a[p, s]) via stride-0 broadcast input
                    exr = eex.tile([128, ns_es, 128], DT.bfloat16, tag="exr")
                    a_ap = a[:]
                    a_bc = AP(a_ap.tensor, a_ap.offset,
                              [a_ap.ap[0], a_ap.ap[1], [0, 128]])
                    nc.scalar.activation(exr[:], a_bc, ACTF.Exp)

                    oh = eoh.tile([128, ns_es, 128], DT.bfloat16, tag="oh")
                    nc.vector.tensor_tensor(oh[:], gw[:, 0:ns_es, 128:256],
                                            exr[:], ALU.mult)

                    # oh pair/quad partial sums so the denominator chain needs
                    # ~ns/4 matmuls instead of ns (PE sequencer relief)
                    np2 = ns_es // 2
                    np4 = np2 // 2
                    ohq_parts = []
                    if np2 > 0:
                        ohp = eoh.tile([128, max(np2, 1), 128], DT.bfloat16,
                                       tag="ohp")
                        nc.vector.tensor_tensor(
                            ohp[:, 0:np2, :], oh[:, 0:2 * np2:2, :],
                            oh[:, 1:2 * np2:2, :], ALU.add)
                        if np4 > 0:
                            ohq = eoh.tile([128, max(np4, 1), 128],
                                           DT.bfloat16, tag="ohq")
                            nc.vector.tensor_tensor(
                                ohq[:, 0:np4, :], ohp[:, 0:2 * np4:2, :],
                                ohp[:, 1:2 * np4:2, :], ALU.add)
                            ohq_parts = [(ohq, s) for s in range(np4)]
                            if np2 > 2 * np4:
                                ohq_parts.append((ohp, np2 - 1))
                        else:
                            ohq_parts = [(ohp, s) for s in range(np2)]
                    if ns_es > 2 * np2:
                        ohq_parts.append((oh, ns_es - 1))

                    pw = eps.tile([128, 128], DT.float32, tag="pw")
                    pwd = eps.tile([128, 1], DT.float32, tag="pwd")
                    for s in range(ns_es):
                        nc.tensor.matmul(pw[:], oh[:, s, :],
                                         gw[:, ns_es + s, 128:256],
                                         start=(s == 0), stop=(s == ns_es - 1),
                                         skip_group_check=True)
                    for i, (t, s) in enumerate(ohq_parts):
                        nc.tensor.matmul(pwd[:], t[:, s, :], ones_sb[:],
                                         start=(i == 0),
                                         stop=(i == len(ohq_parts) - 1),
                                         skip_group_check=True)

                    dn = esm.tile([128, 1], DT.float32, tag="dn")
                    nc.vector.tensor_scalar(dn[:], pwd[:], 1e-30, None, ALU.add)
                    rn = esm.tile([128, 1], DT.float32, tag="rn")
                    nc.vector.reciprocal(rn[:], dn[:])
                    ob = eout.tile([128, 128], DT.bfloat16, tag="ob")
                    if has_bl:
                        o1 = esm.tile([128, 128], DT.float32, tag="o1")
                        nc.vector.scalar_tensor_tensor(
                            o1[:], blin_sb[:], dn[:], pw[:], ALU.mult, ALU.add)
                        nc.scalar.activation(ob[:], o1[:], ACTF.Prelu,
                                             scale=rn[:], alpha=ALPHA)
                    else:
                        nc.scalar.activation(ob[:], pw[:], ACTF.Prelu,
                                             scale=rn[:], alpha=ALPHA)
                    rows_out = WIN if w < nw - 1 else LAST_ROWS
                    nc.sync.dma_start(out_d[w * WIN:w * WIN + rows_out, :],
                                      ob[0:rows_out, :])

    if compile:
        nc.compile()
    return nc


def _store_perm(i):
    """Logical row -> stored row for device-written table chunks.

    Chunk stores write SBUF [128, J, 256] tiles; the DMA lays (partition p,
    subtile j) at chunk row p*J + j while logical row r = j*128 + p."""
    J = CHUNK // 128
    b = i // CHUNK
    r = i % CHUNK
    return b * CHUNK + (r % 128) * J + (r // 128)


def _wrap16(arr):
    """[n] int16 slot-index array (n % 16 == 0) -> [128, n//16] wrapped and
    replicated across the 8 GPSIMD core partition groups."""
    n = arr.shape[0]
    w = arr.reshape(n // 16, 16).T
    return np.tile(w, (8, 1)).astype(np.int16).copy()


def _prep(node, edge_index, W_lin, b_lin, W_att, b_att, a_vec):
    """Host-side sharding/packing. Returns (in_maps, build_key)."""
    recv = np.asarray(edge_index[0], dtype=np.int64)
    send = np.asarray(edge_index[1], dtype=np.int64)

    # ---- fold a_vec into the weights (sign trick) ----
    sa = np.asarray(a_vec[:, 0], dtype=np.float32)
    pos = sa >= 0
    perm = np.concatenate([np.where(pos)[0], np.where(~pos)[0]])
    k = int(pos.sum())
    cvec = np.where(pos, sa, ALPHA * sa).astype(np.float32)

    W_att = np.asarray(W_att, dtype=np.float32)
    w_r_hat = (W_att[:F_DIM, :] * cvec[None, :])[:, perm]
    w_s_hat = (W_att[F_DIM:, :] * cvec[None, :])[:, perm]
    b_hat = (np.asarray(b_att, np.float32) * cvec)[perm]
    has_ba = bool(np.any(b_hat != 0.0))
    b_lin = np.asarray(b_lin, np.float32)
    has_bl = bool(np.any(b_lin != 0.0))

    # ---- edge bucketing ----
    cid = recv // NPC
    rloc = recv - cid * NPC
    wid = rloc >> 7
    grp = (send >= LO_N).astype(np.int64)
    key = (cid * NW + wid) * 2 + grp
    order = np.argsort(key, kind="stable")
    ks, rs, ss = key[order], rloc[order], send[order]
    counts = np.bincount(key, minlength=NCORES * NW * 2).reshape(
        NCORES, NW, 2)
    starts = np.concatenate([[0], np.cumsum(counts.reshape(-1))])[:-1].reshape(
        NCORES, NW, 2)

    ns_lo = np.maximum((counts[:, :, 0].max(axis=0) + 127) // 128, 1)
    ns_hi = np.maximum((counts[:, :, 1].max(axis=0) + 127) // 128, 1)
    wins = tuple((int(ns_lo[w]), int(ns_hi[w])) for w in range(NW))

    # ---- gather index lists ----
    # stored-row mappings
    sp_send_lo = PR_ROWS + _store_perm(np.arange(LO_N))
    sp_send_hi = _store_perm(np.arange(LO_N, N_NODES) - LO_N)

    idx1_c, idx2_c = [], []
    for c in range(NCORES):
        seg1, seg2 = [], []
        for w in range(NW):
            nl, nh = int(ns_lo[w]), int(ns_hi[w])
            ne = nl + nh
            nlo_c, nhi_c = counts[c, w, 0], counts[c, w, 1]
            b0, b1 = starts[c, w, 0], starts[c, w, 1]
            # recv mirror: lo edges (block of nl*128), then hi edges (nh*128)
            rv = np.full(ne * 128, ZROW, np.int64)
            rv[:nlo_c] = rs[b0:b0 + nlo_c]
            rv[nl * 128:nl * 128 + nhi_c] = rs[b1:b1 + nhi_c]
            # pr rows are stored via 4 chunk stores with cols 0:128 =>
            # _store_perm applies to the row index; mask half is written by
            # host at the same permuted row.
            rv = _store_perm(rv)
            # send lo block
            sl = np.full(nl * 128, ZROW, np.int64)
            sl[:nlo_c] = sp_send_lo[ss[b0:b0 + nlo_c]]
            sl[nlo_c:] = _store_perm(ZROW)
            seg1.append(np.concatenate([rv, sl]))
            # send hi block
            sh = np.full(nh * 128, _store_perm(ZROW), np.int64)
            sh[:nhi_c] = sp_send_hi[ss[b1:b1 + nhi_c] - LO_N]
            seg2.append(sh)
        idx1_c.append(np.concatenate(seg1))
        idx2_c.append(np.concatenate(seg2))

    # ---- host tensors ----
    node = np.asarray(node, dtype=np.float32)
    nodeT = np.zeros((128, 25 * CHUNK), BF16)
    nodeT[:, :N_NODES] = node.T.astype(BF16)
    # psw chunk stores permute rows; gather idx maps via _store_perm, so the
    # DRAM table receives rows in *stored* order automatically.  tlo mask
    # half for pr rows must sit at the permuted row position.
    tlo = np.zeros((TLO_ROWS, 256), BF16)
    iperm = _store_perm(np.arange(PR_ROWS))
    logical = np.arange(PR_ROWS)
    valid = logical < ZROW
    eye = np.eye(128, dtype=np.float32).astype(BF16)
    tlo[iperm[valid], 128:256] = eye[logical[valid] % 128]
    thi = np.zeros((THI_ROWS, 256), BF16)

    wcat = np.concatenate([w_s_hat, np.asarray(W_lin, np.float32)],
                          axis=1).astype(BF16)

    in_maps = []
    for c in range(NCORES):
        shardT = np.zeros((128, PR_ROWS), BF16)
        shardT[:, :NPC] = node[c * NPC:(c + 1) * NPC].T.astype(BF16)
        m = {
            "nodeT": nodeT,
            "shardT": shardT,
            "wcat": wcat,
            "wr": w_r_hat.astype(BF16),
            "idx1": _wrap16(idx1_c[c]),
            "idx2": _wrap16(idx2_c[c]),
            "tlo": tlo,
            "thi": thi,
        }
        if has_ba:
            m["bar"] = np.tile(
                np.concatenate([b_hat, np.zeros(128, np.float32)])[None, :],
                (128, 1))
        if has_bl:
            m["blin"] = np.tile(b_lin[None, :], (128, 1))
        in_maps.append(m)
    return in_maps, (wins, k, has_ba, has_bl)


def kernel(node, edge, edge_index, W_lin, b_lin, W_att, b_att, a_vec):
    in_maps, key = _prep(node, edge_index, W_lin, b_lin, W_att, b_att, a_vec)
    if key not in _BUILD_CACHE:
        _BUILD_CACHE[key] = _build(*key)
    nc = _BUILD_CACHE[key]
    res = bass_utils.run_bass_kernel_spmd(nc, in_maps,
                                          core_ids=list(range(NCORES)))
    out = np.concatenate([np.asarray(res.results[c]["out"], np.float32)
                          for c in range(NCORES)], axis=0)
    return np.ascontiguousarray(out[:N_NODES]).astype(np.float32)


# revision 38
# speedup vs baseline: 1.3755x; 1.0976x over previous
"""GATv2 attention head (kgcnn AttentionHeadGATV2) on 8 Trainium2 NeuronCores.

Strategy (edge/graph parallelism, self-contained — no file reads):
  * Host: bucket edges by receiver node; core c owns receivers
    [c*6250, (c+1)*6250), grouped into 49 windows of 128 receivers.
    Within a window edges are split by send row range (lo/hi tables so
    gather indices fit int16) and padded per 128-block to per-window
    capacities (max over the 8 cores, so one SPMD program serves all).
  * Device precompute: two DRAM tables.
      tlo = [ pr-block (8192 rows) | psw rows for send<LO_N ]
      thi = [ psw rows for send>=LO_N ]
    psw row = [P_s | w_n] (256 cols bf16), P_s = node @ (W_att[128:]*c)[:,perm],
    w_n = node @ W_lin;  pr row = [P_r | onehot(row%128)] where
    P_r = node_shard @ (W_att[:128]*c)[:,perm]; the onehot mask half is
    host-initialized, P_r half written by PE matmuls from a transposed
    node input. Column scaling c + permutation fold a_vec into the
    leaky-relu (sign trick) so the logit is a plain row-sum.
  * Edge phase per window w: TWO dma_gathers fill one [128, NS, 256]
    tile: recv block (pr rows: P_r + scatter mask) mirrors the send
    block (psw rows) slot-for-slot.  z = P_r + P_s (DVE);
    h = lrelu sign-split (DVE x2); a = tensor_reduce row-sum (DVE);
    exr = Exp(a) broadcast along units (Act, stride-0 input AP);
    oh = mask * exr (DVE);  PE scatter: pw += oh^T @ w_n, pwd += oh^T @ 1;
    out = Prelu(pw * (1/pwd)) on Act, stored bf16.
  * Padding slots gather table row ZROW (all zeros) => mask 0 => no
    contribution; b_lin / b_att are zero in the graded setup and folded
    via cheap conditional paths when nonzero.
"""

import sys

sys.path.insert(0, "/opt/trn_rl_repo")

import numpy as np
import ml_dtypes

import concourse.bacc as bacc
import concourse.bass as bass
import concourse.mybir as mybir
import concourse.tile as tile
from concourse import bass_utils
from concourse.bass_types import AP

DT = mybir.dt
ALU = mybir.AluOpType
ACTF = mybir.ActivationFunctionType
AXIS = mybir.AxisListType

BF16 = ml_dtypes.bfloat16

# Problem constants (hardcoded per the task contract).
N_NODES = 50000
N_EDGES = 800000
F_DIM = 128
UNITS = 128
ALPHA = 0.2
NCORES = 8
NPC = N_NODES // NCORES          # 6250 receiver nodes per core
WIN = 128                        # receiver-window size (PSUM partition dim)
NW = (NPC + WIN - 1) // WIN      # 49 windows per core
LAST_ROWS = NPC - (NW - 1) * WIN # 106 rows in the last window
CHUNK = 2048                     # precompute chunk rows
PR_ROWS = 8192                   # pr block rows (4 chunks), rows>=6272 zero
ZROW = NW * WIN                  # 6272: all-zero table row for padding slots
LO_N = 18432                     # sends < LO_N gather from tlo (9 chunks)
HI_CHUNKS = 16                   # 16*2048 = 32768 rows >= nodes 18432..50000
TLO_ROWS = PR_ROWS + LO_N        # 26624
THI_ROWS = HI_CHUNKS * CHUNK     # 32768
SCRATCH = 16384                  # SWDGE carveout: 4096 descs per queue
GMAX = 1024                      # max idxs per dma_gather instruction

_BUILD_CACHE = {}


def _build(wins, k_pos, has_ba, has_bl, compile=True):
    """wins: tuple of (ns_lo, ns_hi) per window."""
    nw = len(wins)
    tot16_1 = sum(128 * (2 * (nl + nh) - nh) for nl, nh in wins) // 16
    tot16_2 = sum(128 * nh for nl, nh in wins) // 16

    nc = bacc.Bacc("TRN2", target_bir_lowering=False, debug=False,
                   num_devices=NCORES, num_swdge_queues=4,
                   dynamic_dma_scratch_size=SCRATCH)

    nodeT_d = nc.dram_tensor("nodeT", [128, 25 * CHUNK], DT.bfloat16,
                             kind="ExternalInput")
    shardT_d = nc.dram_tensor("shardT", [128, PR_ROWS], DT.bfloat16,
                              kind="ExternalInput")
    wcat_d = nc.dram_tensor("wcat", [128, 256], DT.bfloat16,
                            kind="ExternalInput")
    wr_d = nc.dram_tensor("wr", [128, 128], DT.bfloat16, kind="ExternalInput")
    idx1_d = nc.dram_tensor("idx1", [128, tot16_1], DT.int16,
                            kind="ExternalInput")
    idx2_d = nc.dram_tensor("idx2", [128, tot16_2], DT.int16,
                            kind="ExternalInput")
    # tlo mask halves (and zero rows) are host-initialized; P_r/psw halves
    # overwritten on device.
    tlo_d = nc.dram_tensor("tlo", [TLO_ROWS, 256], DT.bfloat16,
                           kind="ExternalInput")
    thi_d = nc.dram_tensor("thi", [THI_ROWS, 256], DT.bfloat16,
                           kind="ExternalInput")
    if has_ba:
        bar_d = nc.dram_tensor("bar", [128, 256], DT.float32,
                               kind="ExternalInput")
    if has_bl:
        blin_d = nc.dram_tensor("blin", [128, 128], DT.float32,
                                kind="ExternalInput")
    out_d = nc.dram_tensor("out", [NW * WIN, 128], DT.bfloat16,
                           kind="ExternalOutput")

    with tile.TileContext(nc) as tc:
        with tc.tile_pool(name="consts", bufs=1) as cpool:
            wcat_sb = cpool.tile([128, 256], DT.bfloat16)
            nc.sync.dma_start(wcat_sb[:], wcat_d[:])
            wr_sb = cpool.tile([128, 128], DT.bfloat16)
            nc.sync.dma_start(wr_sb[:], wr_d[:])
            idx1_sb = cpool.tile([128, tot16_1], DT.int16)
            nc.sync.dma_start(idx1_sb[:], idx1_d[:])
            idx2_sb = cpool.tile([128, tot16_2], DT.int16)
            nc.sync.dma_start(idx2_sb[:], idx2_d[:])
            ones_sb = cpool.tile([128, 1], DT.bfloat16)
            nc.vector.memset(ones_sb[:], 1.0)
            if has_ba:
                bar_sb = cpool.tile([128, 256], DT.float32)
                nc.sync.dma_start(bar_sb[:], bar_d[:])
            if has_bl:
                blin_sb = cpool.tile([128, 128], DT.float32)
                nc.sync.dma_start(blin_sb[:], blin_d[:])

            # ---------------- precompute phase ----------------
            J = CHUNK // 128
            with (
                tc.tile_pool(name="pcsb", bufs=3) as pc,
                tc.tile_pool(name="pcps", bufs=4, space="PSUM") as pcp,
            ):
                # pr chunks first so tlo completes as early as possible,
                # letting lo-gathers start while thi is still being written.
                for st in range(PR_ROWS // CHUNK):
                    ntile = pc.tile([128, CHUNK], DT.bfloat16, tag="nT")
                    nc.sync.dma_start(
                        ntile[:], shardT_d[:, st * CHUNK:(st + 1) * CHUNK])
                    rows2 = pc.tile([128, J, 128], DT.bfloat16, tag="rows2")
                    for j4 in range(J // 4):
                        ps = pcp.tile([128, 4, 256], DT.float32, tag="ps")
                        for i in range(4):
                            j = j4 * 4 + i
                            nc.tensor.matmul(ps[:, i, 0:128],
                                             ntile[:, j * 128:(j + 1) * 128],
                                             wr_sb[:], start=True, stop=True)
                        if j4 % 2 == 0:
                            nc.scalar.copy(rows2[:, j4 * 4:(j4 + 1) * 4, :],
                                           ps[:, :, 0:128])
                        else:
                            nc.vector.tensor_scalar(
                                rows2[:, j4 * 4:(j4 + 1) * 4, :],
                                ps[:, :, 0:128], 1.0, None, ALU.mult)
                    r0 = st * CHUNK
                    nc.sync.dma_start(tlo_d[r0:r0 + CHUNK, 0:128], rows2[:])
                # psw chunks: 9 into tlo (after pr block), 16 into thi
                for st in range(9 + HI_CHUNKS):
                    ntile = pc.tile([128, CHUNK], DT.bfloat16, tag="nT")
                    nc.sync.dma_start(
                        ntile[:], nodeT_d[:, st * CHUNK:(st + 1) * CHUNK])
                    rows = pc.tile([128, J, 256], DT.bfloat16, tag="rows")
                    for j4 in range(J // 4):
                        ps = pcp.tile([128, 4, 256], DT.float32, tag="ps")
                        for i in range(4):
                            j = j4 * 4 + i
                            nc.tensor.matmul(ps[:, i, :],
                                             ntile[:, j * 128:(j + 1) * 128],
                                             wcat_sb[:], start=True, stop=True)
                        # PSUM->SBUF convert, alternating engines
                        if has_ba:
                            eng = nc.vector if j4 % 2 == 0 else nc.vector
                            nc.vector.tensor_tensor(
                                rows[:, j4 * 4:(j4 + 1) * 4, :], ps[:],
                                AP(bar_sb[:].tensor, bar_sb[:].offset,
                                   [bar_sb[:].ap[0], [0, 4], bar_sb[:].ap[1]]),
                                ALU.add)
                        elif j4 % 2 == 0:
                            nc.scalar.copy(rows[:, j4 * 4:(j4 + 1) * 4, :],
                                           ps[:])
                        else:
                            nc.vector.tensor_scalar(
                                rows[:, j4 * 4:(j4 + 1) * 4, :], ps[:], 1.0,
                                None, ALU.mult)
                    r0 = st * CHUNK
                    if st < 9:
                        nc.sync.dma_start(
                            tlo_d[PR_ROWS + r0:PR_ROWS + r0 + CHUNK, :],
                            rows[:])
                    else:
                        r0 = (st - 9) * CHUNK
                        nc.sync.dma_start(thi_d[r0:r0 + CHUNK, :], rows[:])

            # ---------------- edge phase ----------------
            with (
                tc.tile_pool(name="egw", bufs=5) as egw,
                tc.tile_pool(name="ez", bufs=2) as ez,
                tc.tile_pool(name="eh", bufs=2) as eh,
                tc.tile_pool(name="eex", bufs=2) as eex,
                tc.tile_pool(name="eoh", bufs=2) as eoh,
                tc.tile_pool(name="esm", bufs=3) as esm,
                tc.tile_pool(name="eout", bufs=2) as eout,
            ):
                off1 = 0
                off2 = 0
                for w in range(nw):
                    ns_lo, ns_hi = wins[w]
                    ns_es = ns_lo + ns_hi
                    NS = 2 * ns_es
                    n1 = 128 * (ns_es + ns_lo)
                    n2 = 128 * ns_hi
                    gw = egw.tile([128, NS, 256], DT.bfloat16, tag="gw")
                    qn = 2 * w
                    for g0 in range(0, n1, GMAX):
                        gn = min(GMAX, n1 - g0)
                        nc.gpsimd.dma_gather(
                            gw[:, g0 // 128:(g0 + gn) // 128, :], tlo_d[:],
                            idx1_sb[:, off1 + g0 // 16:off1 + (g0 + gn) // 16],
                            gn, gn, 256, queue_num=qn % 4)
                        qn += 1
                    base2 = ns_es + ns_lo
                    for g0 in range(0, n2, GMAX):
                        gn = min(GMAX, n2 - g0)
                        nc.gpsimd.dma_gather(
                            gw[:, base2 + g0 // 128:base2 + (g0 + gn) // 128,
                               :], thi_d[:],
                            idx2_sb[:, off2 + g0 // 16:off2 + (g0 + gn) // 16],
                            gn, gn, 256, queue_num=qn % 4)
                        qn += 1
                    off1 += n1 // 16
                    off2 += n2 // 16

                    z = ez.tile([128, ns_es, 128], DT.bfloat16, tag="z")
                    nc.vector.tensor_tensor(z[:], gw[:, 0:ns_es, 0:128],
                                            gw[:, ns_es:NS, 0:128], ALU.add)
                    # leaky-relu sign halves on the Activation engine:
                    # pos block = Prelu(z; 0.2), neg block = Prelu(z; 5.0)
                    h = eh.tile([128, ns_es, 128], DT.bfloat16, tag="h")
                    if k_pos > 0:
                        nc.scalar.activation(h[:, :, 0:k_pos],
                                             z[:, :, 0:k_pos], ACTF.Prelu,
                                             alpha=ALPHA)
                    if k_pos < 128:
                        nc.scalar.activation(h[:, :, k_pos:128],
                                             z[:, :, k_pos:128], ACTF.Prelu,
                                             alpha=1.0 / ALPHA)
                    # bf16 fold tree then fp32 reduce for the row-sum
                    f1 = ez.tile([128, ns_es, 64], DT.bfloat16, tag="f1")
                    nc.vector.tensor_tensor(f1[:], h[:, :, 0:64],
                                            h[:, :, 64:128], ALU.add)
                    f2 = ez.tile([128, ns_es, 32], DT.bfloat16, tag="f2")
                    nc.vector.tensor_tensor(f2[:], f1[:, :, 0:32],
                                            f1[:, :, 32:64], ALU.add)
                    a = esm.tile([128, ns_es], DT.float32, tag="a")
                    nc.vector.tensor_reduce(a[:], f2[:], AXIS.X, ALU.add)

                    # exr[p, s, :] = exp(a[p, s]) via stride-0 broadcast input
                    exr = eex.tile([128, ns_es, 128], DT.bfloat16, tag="exr")
                    a_ap = a[:]
                    a_bc = AP(a_ap.tensor, a_ap.offset,
                              [a_ap.ap[0], a_ap.ap[1], [0, 128]])
                    nc.scalar.activation(exr[:], a_bc, ACTF.Exp)

                    oh = eoh.tile([128, ns_es, 128], DT.bfloat16, tag="oh")
                    nc.vector.tensor_tensor(oh[:], gw[:, 0:ns_es, 128:256],
                                            exr[:], ALU.mult)

                    pw = eps.tile([128, 128], DT.float32, tag="pw")
                    pwd = eps.tile([128, 1], DT.float32, tag="pwd")
                    for s in range(ns_es):
                        nc.tensor.matmul(pw[:], oh[:, s, :],
                                         gw[:, ns_es + s, 128:256],
                                         start=(s == 0), stop=(s == ns_es - 1),
                                         skip_group_check=True)
                        nc.tensor.matmul(pwd[:], oh[:, s, :], ones_sb[:],
                                         start=(s == 0), stop=(s == ns_es - 1),
                                         skip_group_check=True)

                    dn = esm.tile([128, 1], DT.float32, tag="dn")
                    nc.vector.tensor_scalar(dn[:], pwd[:], 1e-30, None, ALU.add)
                    rn = esm.tile([128, 1], DT.float32, tag="rn")
                    nc.vector.reciprocal(rn[:], dn[:])
                    # batch output stores 4 windows per DMA; the [128,4,128]
                    # tile lands in DRAM as row = pos*4 + (w%4) within the
                    # group (host maps rows accordingly).
                    if w % 4 == 0:
                        nw_grp = min(4, nw - w)
                        ob = eout.tile([128, nw_grp, 128], DT.bfloat16,
                                       tag="ob")
                    tgt = ob[:, w % 4, :]
                    if has_bl:
                        o1 = esm.tile([128, 128], DT.float32, tag="o1")
                        nc.vector.scalar_tensor_tensor(
                            o1[:], blin_sb[:], dn[:], pw[:], ALU.mult, ALU.add)
                        nc.scalar.activation(tgt, o1[:], ACTF.Prelu,
                                             scale=rn[:], alpha=ALPHA)
                    else:
                        nc.scalar.activation(tgt, pw[:], ACTF.Prelu,
                                             scale=rn[:], alpha=ALPHA)
                    if w % 4 == 3 or w == nw - 1:
                        w0 = w - w % 4
                        nc.sync.dma_start(
                            out_d[w0 * WIN:w0 * WIN + nw_grp * WIN, :], ob[:])

    if compile:
        nc.compile()
    return nc


def _store_perm(i):
    """Logical row -> stored row for device-written table chunks.

    Chunk stores write SBUF [128, J, 256] tiles; the DMA lays (partition p,
    subtile j) at chunk row p*J + j while logical row r = j*128 + p."""
    J = CHUNK // 128
    b = i // CHUNK
    r = i % CHUNK
    return b * CHUNK + (r % 128) * J + (r // 128)


def _wrap16(arr):
    """[n] int16 slot-index array (n % 16 == 0) -> [128, n//16] wrapped and
    replicated across the 8 GPSIMD core partition groups."""
    n = arr.shape[0]
    w = arr.reshape(n // 16, 16).T
    return np.tile(w, (8, 1)).astype(np.int16).copy()


def _pack_windows(deg_lo, deg_hi):
    """Pack the N receiver nodes into NCORES*NW windows of <=128 nodes,
    balancing per-window lo/hi edge counts toward multiples of 128, then
    group windows into NW slots of NCORES with matching block budgets.

    Returns (node2core, node2slot, node2pos): per-node placement."""
    nwin = NCORES * NW
    n_nodes = len(deg_lo)

    # serpentine deal by hi-degree: near-uniform per-window hi/lo sums
    order = np.argsort(-(deg_hi * 4096 + deg_lo), kind="stable")
    win_of = np.empty(n_nodes, np.int64)
    for r in range(0, n_nodes, nwin):
        idx = order[r:r + nwin]
        cols = np.arange(len(idx))
        if (r // nwin) % 2 == 1:
            cols = len(idx) - 1 - cols
        win_of[idx] = cols
    lo_sum = np.bincount(win_of, weights=deg_lo, minlength=nwin)
    hi_sum = np.bincount(win_of, weights=deg_hi, minlength=nwin)

    # repair: push most windows under b*128 block boundaries by swapping
    # high-degree nodes into a small set of designated "big" windows.
    members = [list(np.where(win_of == w)[0]) for w in range(nwin)]

    def _repair(deg, sums, other_deg, other_sums, cap, bigcap, nbig):
        big = list(np.argsort(-sums)[:nbig])
        bigset = set(big)
        for w in range(nwin):
            if w in bigset:
                continue
            guard = 0
            while sums[w] > cap and guard < 20:
                guard += 1
                # node with max deg in w
                a = max(members[w], key=lambda n: deg[n])
                cands = [b for b in big if sums[b] + deg[a] <= bigcap]
                if not cands:
                    break
                b_w = max(cands, key=lambda b: bigcap - sums[b])
                # victim: smallest deg in big window with similar other-deg
                v = min(members[b_w],
                        key=lambda n: deg[n] * 1000 +
                        abs(other_deg[n] - other_deg[a]))
                if deg[v] >= deg[a]:
                    break
                members[w].remove(a)
                members[b_w].remove(v)
                members[w].append(v)
                members[b_w].append(a)
                sums[w] += deg[v] - deg[a]
                sums[b_w] += deg[a] - deg[v]
                other_sums[w] += other_deg[v] - other_deg[a]
                other_sums[b_w] += other_deg[a] - other_deg[v]

    _repair(deg_hi, hi_sum, deg_lo, lo_sum, 1280.0, 1408.0, 40)
    _repair(deg_lo, lo_sum, deg_hi, hi_sum, 768.0, 896.0, 40)
    for w in range(nwin):
        for n in members[w]:
            win_of[n] = w
    lo_sum = np.bincount(win_of, weights=deg_lo, minlength=nwin)
    hi_sum = np.bincount(win_of, weights=deg_hi, minlength=nwin)
    wkey = np.lexsort((hi_sum, np.ceil(hi_sum / 128), np.ceil(lo_sum / 128)))
    slot_of_win = np.empty(nwin, np.int64)
    core_of_win = np.empty(nwin, np.int64)
    for s in range(NW):
        grp = wkey[s * NCORES:(s + 1) * NCORES]
        slot_of_win[grp] = s
        core_of_win[grp] = np.arange(NCORES)
    node2core = core_of_win[win_of]
    node2slot = slot_of_win[win_of]
    # position within window: stable order by node id
    key = node2core * NW + node2slot
    order2 = np.argsort(key, kind="stable")
    pos = np.empty(len(key), np.int64)
    idx = np.arange(len(key))
    boundaries = np.concatenate([[0], np.cumsum(np.bincount(
        key, minlength=nwin))])
    pos[order2] = idx - boundaries[key[order2]]
    return node2core, node2slot, pos


def _prep(node, edge_index, W_lin, b_lin, W_att, b_att, a_vec):
    """Host-side sharding/packing. Returns (in_maps, build_key, extras)."""
    recv = np.asarray(edge_index[0], dtype=np.int64)
    send = np.asarray(edge_index[1], dtype=np.int64)

    # ---- fold a_vec into the weights (sign trick) ----
    sa = np.asarray(a_vec[:, 0], dtype=np.float32)
    pos = sa >= 0
    perm = np.concatenate([np.where(pos)[0], np.where(~pos)[0]])
    k = int(pos.sum())
    cvec = np.where(pos, sa, ALPHA * sa).astype(np.float32)

    W_att = np.asarray(W_att, dtype=np.float32)
    w_r_hat = (W_att[:F_DIM, :] * cvec[None, :])[:, perm]
    w_s_hat = (W_att[F_DIM:, :] * cvec[None, :])[:, perm]
    b_hat = (np.asarray(b_att, np.float32) * cvec)[perm]
    has_ba = bool(np.any(b_hat != 0.0))
    b_lin = np.asarray(b_lin, np.float32)
    has_bl = bool(np.any(b_lin != 0.0))

    # ---- receiver-window packing ----
    grp_e = (send >= LO_N).astype(np.int64)
    deg_lo = np.bincount(recv[grp_e == 0], minlength=N_NODES)
    deg_hi = np.bincount(recv[grp_e == 1], minlength=N_NODES)
    node2core, node2slot, node2pos = _pack_windows(deg_lo, deg_hi)

    # ---- edge bucketing (pass 1: slot sizes) ----
    def _bucket():
        cid = node2core[recv]
        wid = node2slot[recv]
        key = (cid * NW + wid) * 2 + grp_e
        order = np.argsort(key, kind="stable")
        rloc_all = node2slot[recv] * WIN + node2pos[recv]
        ks, rs, ss = key[order], rloc_all[order], send[order]
        counts = np.bincount(key, minlength=NCORES * NW * 2).reshape(
            NCORES, NW, 2)
        starts = np.concatenate(
            [[0], np.cumsum(counts.reshape(-1))])[:-1].reshape(NCORES, NW, 2)
        ns_lo = np.maximum((counts[:, :, 0].max(axis=0) + 127) // 128, 1)
        ns_hi = np.maximum((counts[:, :, 1].max(axis=0) + 127) // 128, 1)
        return rs, ss, counts, starts, ns_lo, ns_hi

    rs, ss, counts, starts, ns_lo, ns_hi = _bucket()
    # pyramid processing order: small windows at both ends (fast pipeline
    # fill at the start, short compute drain at the end), big in the middle
    asc = np.argsort(ns_lo + ns_hi, kind="stable")
    order_slots = np.concatenate([asc[0::2], asc[1::2][::-1]])
    slot_rank = np.empty(NW, np.int64)
    slot_rank[order_slots] = np.arange(NW)
    node2slot = slot_rank[node2slot]
    rs, ss, counts, starts, ns_lo, ns_hi = _bucket()
    wins = tuple((int(ns_lo[w]), int(ns_hi[w])) for w in range(NW))

    # ---- gather index lists ----
    # stored-row mappings
    sp_send_lo = PR_ROWS + _store_perm(np.arange(LO_N))
    sp_send_hi = _store_perm(np.arange(LO_N, N_NODES) - LO_N)

    idx1_c, idx2_c = [], []
    for c in range(NCORES):
        seg1, seg2 = [], []
        for w in range(NW):
            nl, nh = int(ns_lo[w]), int(ns_hi[w])
            ne = nl + nh
            nlo_c, nhi_c = counts[c, w, 0], counts[c, w, 1]
            b0, b1 = starts[c, w, 0], starts[c, w, 1]
            # recv mirror: lo edges (block of nl*128), then hi edges (nh*128)
            rv = np.full(ne * 128, ZROW, np.int64)
            rv[:nlo_c] = rs[b0:b0 + nlo_c]
            rv[nl * 128:nl * 128 + nhi_c] = rs[b1:b1 + nhi_c]
            # pr rows are stored via 4 chunk stores with cols 0:128 =>
            # _store_perm applies to the row index; mask half is written by
            # host at the same permuted row.
            rv = _store_perm(rv)
            # send lo block
            sl = np.full(nl * 128, ZROW, np.int64)
            sl[:nlo_c] = sp_send_lo[ss[b0:b0 + nlo_c]]
            sl[nlo_c:] = _store_perm(ZROW)
            seg1.append(np.concatenate([rv, sl]))
            # send hi block
            sh = np.full(nh * 128, _store_perm(ZROW), np.int64)
            sh[:nhi_c] = sp_send_hi[ss[b1:b1 + nhi_c] - LO_N]
            seg2.append(sh)
        idx1_c.append(np.concatenate(seg1))
        idx2_c.append(np.concatenate(seg2))

    # ---- host tensors ----
    node = np.asarray(node, dtype=np.float32)
    nodeT = np.zeros((128, 25 * CHUNK), BF16)
    nodeT[:, :N_NODES] = node.T.astype(BF16)
    # psw chunk stores permute rows; gather idx maps via _store_perm, so the
    # DRAM table receives rows in *stored* order automatically.  tlo mask
    # half for pr rows must sit at the permuted row position.
    tlo = np.zeros((TLO_ROWS, 256), BF16)
    iperm = _store_perm(np.arange(PR_ROWS))
    logical = np.arange(PR_ROWS)
    valid = logical < ZROW
    eye = np.eye(128, dtype=np.float32).astype(BF16)
    tlo[iperm[valid], 128:256] = eye[logical[valid] % 128]
    thi = np.zeros((THI_ROWS, 256), BF16)

    wcat = np.concatenate([w_s_hat, np.asarray(W_lin, np.float32)],
                          axis=1).astype(BF16)

    prrow = node2slot * WIN + node2pos
    in_maps = []
    for c in range(NCORES):
        shardT = np.zeros((128, PR_ROWS), BF16)
        sel = node2core == c
        shardT[:, prrow[sel]] = node[sel].T.astype(BF16)
        m = {
            "nodeT": nodeT,
            "shardT": shardT,
            "wcat": wcat,
            "wr": w_r_hat.astype(BF16),
            "idx1": _wrap16(idx1_c[c]),
            "idx2": _wrap16(idx2_c[c]),
            "tlo": tlo,
            "thi": thi,
        }
        if has_ba:
            m["bar"] = np.tile(
                np.concatenate([b_hat, np.zeros(128, np.float32)])[None, :],
                (128, 1))
        if has_bl:
            m["blin"] = np.tile(b_lin[None, :], (128, 1))
        in_maps.append(m)
    # out rows: stores batch 4 windows per DMA -> within group g=slot//4 the
    # DRAM row is pos*gsz + slot%4 (gsz = windows in the group)
    g = node2slot // 4
    gsz = np.minimum(4, NW - g * 4)
    out_row = g * 512 + node2pos * gsz + (node2slot % 4)
    return in_maps, (wins, k, has_ba, has_bl), (node2core, out_row)


def kernel(node, edge, edge_index, W_lin, b_lin, W_att, b_att, a_vec):
    in_maps, key, place = _prep(node, edge_index, W_lin, b_lin, W_att, b_att,
                                a_vec)
    if key not in _BUILD_CACHE:
        _BUILD_CACHE[key] = _build(*key)
    nc = _BUILD_CACHE[key]
    res = bass_utils.run_bass_kernel_spmd(nc, in_maps,
                                          core_ids=list(range(NCORES)))
    node2core, prrow = place
    outs = np.stack([np.asarray(res.results[c]["out"], np.float32)
                     for c in range(NCORES)], axis=0)
    return np.ascontiguousarray(outs[node2core, prrow, :]).astype(np.float32)
